# revision 1
# baseline (speedup 1.0000x reference)
"""BlockSparseMLP (MoE top-2 routing, 8 experts) — Trainium2 Bass kernel.

Strategy (expert-parallel, per sharding hint): one expert per NeuronCore.
Host-side (numpy): router (x @ gate_tensor, softmax, top-2, renormalize),
token dispatch (gather the tokens routed to each expert, transposed to
feature-major and pre-swizzled into the SBUF block layout), and the final
combine (scatter-add of the per-expert partial outputs) — the
shard/unshard stage.

Device-side (one SPMD Bass/Tile program on 8 cores): the expert gated MLP
   gT = Wg_e.T @ xT_e   (bf16 matmuls, fp32 PSUM accumulate)
   uT = Wu_e.T @ xT_e
   aT = silu(gT) * uT   (bf16 in SBUF)
   dT = Wd_e.T @ aT
   outT = dT * w_e      (combine weights folded in on-chip)

All tensors are cast fp32 -> bf16 inside the DMA datapath (SWDGE cast):
HBM traffic stays the input's fp32 bytes but SBUF/PE run bf16.  Weights
are pre-swizzled on the host into per-DMA-block partition-major layout so
every SWDGE transfer reads large contiguous chunks (full descriptor
efficiency); the token axis is the matmul moving dimension, split into
chunks <= 512 (PSUM bank limit).
"""

import os

import numpy as np

T, D, F, E, TOPK = 2048, 2048, 5632, 8, 2
P = 128
KD = D // P     # 16 k-subtiles over D
KF = F // P     # 44 k-subtiles over F
FG = 4          # f-tiles per phase-1 weight DMA block (512 F columns)
NFG = KF // FG  # 11 phase-1 blocks
DG = 2          # d-tiles per phase-2 psum group (256 D columns)
NDG = KD // DG  # 8 phase-2 d-groups
KO2 = 4         # f-subtiles per phase-2 weight DMA block
NFB = KF // KO2  # 11 phase-2 blocks per d-group

_COMPILED = {}   # CAP -> (nc, chunk list)
LAST_RESULT = None  # BassKernelResults of the most recent run (for test.py)


def _token_chunks(cap):
    """Split cap into free-dim chunks, each in [256, 512]."""
    assert cap >= 512 and cap % 2 == 0
    n512, rem = divmod(cap, 512)
    if rem == 0:
        return [512] * n512
    if rem >= 256:
        return [512] * n512 + [rem]
    return [512] * (n512 - 1) + [256 + rem, 256]


def _build(cap):
    """Build + compile the SPMD Tile program for token capacity `cap`."""
    import concourse.bass as bass  # noqa: F401
    import concourse.mybir as mybir
    import concourse.tile as tile
    from concourse import bacc

    f32 = mybir.dt.float32
    bf16 = mybir.dt.bfloat16
    mult = mybir.AluOpType.mult

    chunks = _token_chunks(cap)
    starts = [sum(chunks[:i]) for i in range(len(chunks))]

    nc = bacc.Bacc("TRN2", target_bir_lowering=False, debug=False,
                   enable_asserts=False, num_devices=E)

    xt_d = nc.dram_tensor("xt", [P, KD, cap], f32, kind="ExternalInput").ap()
    wg_d = nc.dram_tensor("wg", [NFG, P, KD, P * FG], f32,
                          kind="ExternalInput").ap()
    wu_d = nc.dram_tensor("wu", [NFG, P, KD, P * FG], f32,
                          kind="ExternalInput").ap()
    wd_d = nc.dram_tensor("wd", [NDG, NFB, P, KO2, P * DG], f32,
                          kind="ExternalInput").ap()
    wr_d = nc.dram_tensor("wrep", [P, cap], f32, kind="ExternalInput").ap()
    out_d = nc.dram_tensor("out_t", [D, cap], f32, kind="ExternalOutput").ap()
    scr_d = nc.dram_tensor("scr", [P, 512], f32).ap()   # warm-up sink

    with tile.TileContext(nc) as tc:
        with (
            tc.tile_pool(name="resident", bufs=1) as rpool,
            tc.tile_pool(name="w1", bufs=3) as w1pool,
            tc.tile_pool(name="wd2", bufs=8) as wd2pool,
            tc.tile_pool(name="outp", bufs=4) as outpool,
            tc.tile_pool(name="psum", bufs=2, space="PSUM") as ppool,
        ):
            xt = rpool.tile([P, KD, cap], bf16)
            wrep = rpool.tile([P, cap], f32)
            nc.sync.dma_start(wrep[:], wr_d)
            at = rpool.tile([P, KF, cap], bf16)

            # Warm-up: the first real matmul can't start until ~17us (DMA
            # latency).  Run throwaway matmuls on a zeroed tile during that
            # window so the PE HAM clock-gate opens (1.2 -> 2.4 GHz) before
            # real work arrives, and the transition timing is deterministic.
            warm = rpool.tile([P, 512], bf16)
            nc.vector.memset(warm[:], 0.0)
            wps = ppool.tile([P, 512], f32, tag="ps0c0", name="warm_ps")
            for i in range(20):
                nc.tensor.matmul(wps[:], warm[:, :P], warm[:],
                                 start=(i == 0), stop=(i == 19))
            wout = rpool.tile([P, 512], f32)
            nc.vector.tensor_copy(out=wout[:], in_=wps[:])
            nc.sync.dma_start(scr_d[:], wout[:])

            # Queue order on the single SWDGE ring decides arrival order:
            # first weight sub-block + first token slices (so PE can start
            # ~13us in), then the token bulk, then the stream.
            nc.gpsimd.dma_start(xt[:, :2, :], xt_d[:, :2, :])

            w1tiles = []
            for fg in range(NFG):
                wgb = w1pool.tile([P, KD, P * FG], bf16, tag="wgb",
                                  name=f"wgb_{fg}")
                wub = w1pool.tile([P, KD, P * FG], bf16, tag="wub",
                                  name=f"wub_{fg}")
                w1tiles.append((wgb, wub))
                if fg == 0:
                    # fine-grained first block + token bulk spread over
                    # several DMAs so multiple SWDGE lanes pull in parallel
                    for s in range(FG):
                        sl = slice(s * P, (s + 1) * P)
                        nc.gpsimd.dma_start(wgb[:, :, sl], wg_d[0][:, :, sl])
                        nc.gpsimd.dma_start(wub[:, :, sl], wu_d[0][:, :, sl])
                        if s == 0:
                            for k0 in range(2, KD, 2):
                                nc.gpsimd.dma_start(
                                    xt[:, k0:k0 + 2, :], xt_d[:, k0:k0 + 2, :])
                else:
                    kh = KD // 2
                    nc.gpsimd.dma_start(wgb[:, :kh, :], wg_d[fg][:, :kh, :])
                    nc.gpsimd.dma_start(wgb[:, kh:, :], wg_d[fg][:, kh:, :])
                    nc.gpsimd.dma_start(wub[:, :kh, :], wu_d[fg][:, :kh, :])
                    nc.gpsimd.dma_start(wub[:, kh:, :], wu_d[fg][:, kh:, :])

                # ---- phase 1: gT/uT = W.T @ xT, aT = silu(gT)*uT ----
                for fs in range(FG):
                    ft = fg * FG + fs
                    for ci, (c0, cn) in enumerate(zip(starts, chunks)):
                        pg = ppool.tile([P, cn], f32, tag=f"ps0c{ci}")
                        pu = ppool.tile([P, cn], f32, tag=f"ps1c{ci}")
                        for ko in range(KD):
                            nc.tensor.matmul(
                                pg[:], wgb[:, ko, fs * P:(fs + 1) * P],
                                xt[:, ko, c0:c0 + cn],
                                start=(ko == 0), stop=(ko == KD - 1))
                        for ko in range(KD):
                            nc.tensor.matmul(
                                pu[:], wub[:, ko, fs * P:(fs + 1) * P],
                                xt[:, ko, c0:c0 + cn],
                                start=(ko == 0), stop=(ko == KD - 1))
                        a_sl = at[:, ft, c0:c0 + cn]
                        nc.scalar.activation(
                            a_sl, pg[:], mybir.ActivationFunctionType.Silu)
                        nc.vector.tensor_tensor(a_sl, a_sl, pu[:], mult)

            # ---- phase 2: dT = Wd.T @ aT, out = dT * w ----
            for dg in range(NDG):
                pds = [[ppool.tile([P, cn], f32, tag=f"ps{ds}c{ci}",
                                   name=f"pd_{dg}_{ds}_{ci}")
                        for ci, cn in enumerate(chunks)]
                       for ds in range(DG)]
                for fb in range(NFB):
                    wdb = wd2pool.tile([P, KO2, P * DG], bf16, tag="wdb")
                    nc.gpsimd.dma_start(wdb[:], wd_d[dg, fb])
                    for ko in range(KO2):
                        fk = fb * KO2 + ko
                        for ds in range(DG):
                            for ci, (c0, cn) in enumerate(zip(starts, chunks)):
                                nc.tensor.matmul(
                                    pds[ds][ci][:],
                                    wdb[:, ko, ds * P:(ds + 1) * P],
                                    at[:, fk, c0:c0 + cn],
                                    start=(fk == 0), stop=(fk == KF - 1))
                for ds in range(DG):
                    ot = outpool.tile([P, cap], f32, tag="ot")
                    for ci, (c0, cn) in enumerate(zip(starts, chunks)):
                        nc.vector.tensor_tensor(
                            ot[:, c0:c0 + cn], pds[ds][ci][:],
                            wrep[:, c0:c0 + cn], mult)
                    dt_idx = dg * DG + ds
                    nc.sync.dma_start(
                        out_d[dt_idx * P:(dt_idx + 1) * P, :], ot[:])

    nc.compile()
    return nc, chunks


def _swizzle_w1(w):
    """[D, F] -> [NFG, P, KD, P*FG] block-major, partition-contiguous."""
    return np.ascontiguousarray(
        w.reshape(KD, P, NFG, P * FG).transpose(2, 1, 0, 3))


def _swizzle_wd(w):
    """[F, D] -> [NDG, NFB, P, KO2, P*DG] block-major."""
    return np.ascontiguousarray(
        w.reshape(NFB, KO2, P, NDG, P * DG).transpose(3, 0, 2, 1, 4))


def kernel(x, gate_tensor, Wg, Wu, Wd):
    global LAST_RESULT
    from concourse.bass_interp import get_hw_module
    from concourse.bass_utils import run_bass_kernel_spmd

    x = np.ascontiguousarray(np.asarray(x, dtype=np.float32))
    gate_tensor = np.asarray(gate_tensor, dtype=np.float32)
    Wg = np.asarray(Wg, dtype=np.float32)
    Wu = np.asarray(Wu, dtype=np.float32)
    Wd = np.asarray(Wd, dtype=np.float32)

    # ---- router (replicated; tiny: T*D*E flops) ----
    logits = x @ gate_tensor                      # [T, E] fp32
    m = logits.max(axis=-1, keepdims=True)
    p = np.exp(logits - m, dtype=np.float32)
    p /= p.sum(axis=-1, keepdims=True)
    topi = np.argsort(-p, axis=-1, kind="stable")[:, :TOPK]      # [T, K]
    topw = np.take_along_axis(p, topi, axis=-1)
    topw = topw / (topw.sum(axis=-1, keepdims=True) + 1e-20)

    idx = []          # tokens routed to each expert
    wts = []          # their combine weights
    for e in range(E):
        sel = (topi == e)                         # [T, K]; <=1 True per row
        idx.append(np.nonzero(sel.any(axis=-1))[0])
        wts.append(topw[sel].astype(np.float32))  # row-major == token order
    max_n = max(len(t) for t in idx)
    cap = max(512, ((max_n + 1) // 2) * 2)

    if cap not in _COMPILED:
        _COMPILED[cap] = _build(cap)
    nc, _chunks = _COMPILED[cap]

    # ---- dispatch: per-core inputs (pre-swizzled to SBUF block layout) ----
    in_maps = []
    for e in range(E):
        n = len(idx[e])
        xg = x[idx[e]]                            # [n, D]
        xt = np.zeros((P, KD, cap), dtype=np.float32)
        xt[:, :, :n] = xg.T.reshape(KD, P, n).transpose(1, 0, 2)
        wr = np.zeros((P, cap), dtype=np.float32)
        wr[:, :n] = wts[e][None, :]
        in_maps.append({"xt": xt, "wg": _swizzle_w1(Wg[e]),
                        "wu": _swizzle_w1(Wu[e]), "wd": _swizzle_wd(Wd[e]),
                        "wrep": wr})

    trace = bool(int(os.environ.get("KERNEL_TRACE", "0")))
    old_m = nc.m
    nc.m = get_hw_module(nc.m)
    try:
        try:
            res = run_bass_kernel_spmd(nc, in_maps, core_ids=list(range(E)),
                                       trace=trace)
        except (ImportError, ModuleNotFoundError):
            # tracing requested (e.g. BASS_TRACE in the env) but this image
            # lacks the axon NTFF profile hook -- rerun without tracing
            os.environ["BASS_NEVER_TRACE"] = "1"
            res = run_bass_kernel_spmd(nc, in_maps, core_ids=list(range(E)),
                                       trace=False)
    finally:
        nc.m = old_m
    LAST_RESULT = res

    # ---- combine: scatter-add the per-expert partials ----
    out = np.zeros((T, D), dtype=np.float32)
    for e in range(E):
        n = len(idx[e])
        out[idx[e]] += res.results[e]["out_t"][:, :n].T
    return out



# revision 2
# speedup vs baseline: 1.0078x; 1.0078x over previous
"""BlockSparseMLP (MoE top-2 routing, 8 experts) — Trainium2 Bass kernel.

Strategy v2 (expert-group tensor-parallel, "TP-4"):

The 8 experts are split into 2 groups of 4, chosen to balance total token
load across groups (sorted-alternating assignment).  Each group owns 4
cores; core (g, q) holds the q-th F-quarter (1408 = 11 f-tiles) of all 4
experts in group g and computes, for every token routed to those experts,
the gated-MLP over its quarter:

   gT = Wg[e][:, q].T @ xT      (bf16 matmuls, fp32 PSUM)
   uT = Wu[e][:, q].T @ xT
   aT = silu(gT) * uT
   dT_partial = Wd[e][q, :].T @ aT      (partial over the F-quarter)
   outT += dT_partial * w_e             (combine weight folded on-chip)

Host sums the 4 partial outputs of each group (the tp_reduce) and
scatter-adds into the final [T, D] output.

Why: with one-expert-per-core SPMD, every core pays the *maximum* expert
load (545 tokens for seed 0).  With 4-way F-parallelism over expert
groups, a core processes its group's token total over a quarter of F;
balanced groups bring the per-core work to ~2064 token-quarters = 516
token-equivalents, recovering the load-imbalance loss and cutting the
instruction count (11 f-tiles per phase-1 pass instead of 44).

All inputs are cast to bf16 on the host and uploaded as bf16 — HBM
traffic halves vs fp32+SWDGE-cast, which removes the startup weight-
stream starvation the fp32 version showed in traces.
"""

import os

import numpy as np

T, D, F, E, TOPK = 2048, 2048, 5632, 8, 2
P = 128
KD = D // P          # 16 k-subtiles over D
NQ = 4               # cores per expert group (F quartering)
FQ = F // NQ         # 1408 F-columns per core
KFQ = FQ // P        # 11 f-tiles per core
NFB = 3              # phase-1 weight DMA blocks per (expert, matrix)
FBW = 512            # f-columns per block (last block: 384 real + 128 pad)
FT_PER_FB = (4, 4, 3)
NDG = 8              # phase-2 d-groups (256 D-columns each)
DG = 2               # d-tiles per d-group

_COMPILED = {}       # (slot_sizes, chunks) -> nc
LAST_RESULT = None   # BassKernelResults of the most recent run (for test.py)


def _slot_chunks(s):
    """Split slot size s into <=512 even-sized chunks."""
    nch = -(-s // 512)
    base = s // nch
    sizes = [base] * nch
    for i in range(s - base * nch):
        sizes[i] += 1
    # make sizes even where possible (s is even)
    for i in range(0, len(sizes) - 1):
        if sizes[i] % 2:
            sizes[i] += 1
            sizes[i + 1] -= 1
    return sizes


def _build(slot_sizes):
    """Build + compile the SPMD Tile program for per-slot token counts."""
    import concourse.bass as bass  # noqa: F401
    import concourse.mybir as mybir
    import concourse.tile as tile
    from concourse import bacc

    f32 = mybir.dt.float32
    bf16 = mybir.dt.bfloat16
    mult = mybir.AluOpType.mult

    nslot = len(slot_sizes)
    slotmax = max(slot_sizes)
    cap = sum(slot_sizes)
    soffs = [sum(slot_sizes[:j]) for j in range(nslot)]
    chunks = [_slot_chunks(s) for s in slot_sizes]
    nch_max = max(len(c) for c in chunks)
    psum_bufs = 2 if nch_max <= 2 else 1

    nc = bacc.Bacc("TRN2", target_bir_lowering=False, debug=False,
                   enable_asserts=False, num_devices=E)

    xt_d = nc.dram_tensor("xt", [P, KD, cap], bf16, kind="ExternalInput").ap()
    wg_d = nc.dram_tensor("wg", [nslot, NFB, P, KD, FBW], bf16,
                          kind="ExternalInput").ap()
    wu_d = nc.dram_tensor("wu", [nslot, NFB, P, KD, FBW], bf16,
                          kind="ExternalInput").ap()
    wd_d = nc.dram_tensor("wd", [nslot, NDG, P, KFQ, P * DG], bf16,
                          kind="ExternalInput").ap()
    wr_d = nc.dram_tensor("wrep", [P, cap], f32, kind="ExternalInput").ap()
    out_d = nc.dram_tensor("out_t", [D, cap], f32, kind="ExternalOutput").ap()
    scr_d = nc.dram_tensor("scr", [P, 512], f32).ap()   # warm-up sink

    with tile.TileContext(nc) as tc:
        with (
            tc.tile_pool(name="resident", bufs=1) as rpool,
            tc.tile_pool(name="atp", bufs=2) as atpool,
            tc.tile_pool(name="w1", bufs=2) as w1pool,
            tc.tile_pool(name="wd2", bufs=4) as wd2pool,
            tc.tile_pool(name="outp", bufs=4) as outpool,
            tc.tile_pool(name="psum", bufs=psum_bufs, space="PSUM") as ppool,
        ):
            xt = rpool.tile([P, KD, cap], bf16)
            wrep = rpool.tile([P, cap], f32)
            nc.sync.dma_start(wrep[:], wr_d)

            # Warm-up: run throwaway matmuls on a zeroed tile while the
            # first weight/token DMAs are in flight, so the PE HAM
            # clock-gate opens (1.2 -> 2.4 GHz) before real work arrives.
            warm = rpool.tile([P, 512], bf16)
            nc.vector.memset(warm[:], 0.0)
            wps = ppool.tile([P, 512], f32, tag="g0", name="warm_ps")
            for i in range(10):
                nc.tensor.matmul(wps[:], warm[:, :P], warm[:],
                                 start=(i == 0), stop=(i == 9))
            wout = rpool.tile([P, 512], f32)
            nc.vector.tensor_copy(out=wout[:], in_=wps[:])
            nc.sync.dma_start(scr_d[:], wout[:])

            # ---- token DMA, interleaved with slot-0's first weight block
            # (finely sliced so the PE can start as early as possible) ----
            nc.gpsimd.dma_start(xt[:, :2, :], xt_d[:, :2, :])

            w1tiles = [[None] * NFB for _ in range(nslot)]
            wgb0 = w1pool.tile([P, KD, FBW], bf16, tag="wg", name="wgb_0_0")
            wub0 = w1pool.tile([P, KD, FBW], bf16, tag="wu", name="wub_0_0")
            w1tiles[0][0] = (wgb0, wub0)
            for s in range(4):
                sl = slice(s * P, (s + 1) * P)
                nc.gpsimd.dma_start(wgb0[:, :, sl], wg_d[0, 0][:, :, sl])
                nc.gpsimd.dma_start(wub0[:, :, sl], wu_d[0, 0][:, :, sl])
                if s < 3:
                    k0 = 2 + s * 4
                    nc.gpsimd.dma_start(xt[:, k0:k0 + 4, :],
                                        xt_d[:, k0:k0 + 4, :])
                else:
                    nc.gpsimd.dma_start(xt[:, 14:, :], xt_d[:, 14:, :])

            at_tiles = [None] * nslot

            for j in range(nslot):
                soff = soffs[j]
                jchunks = chunks[j]
                jstarts = [sum(jchunks[:i]) for i in range(len(jchunks))]
                at = atpool.tile([P, KFQ, slotmax], bf16, tag="at",
                                 name=f"at_{j}")
                at_tiles[j] = at

                # ---- phase 1: gT/uT = W.T @ xT, aT = silu(gT)*uT ----
                for fb in range(NFB):
                    if w1tiles[j][fb] is None:
                        wgb = w1pool.tile([P, KD, FBW], bf16, tag="wg",
                                          name=f"wgb_{j}_{fb}")
                        wub = w1pool.tile([P, KD, FBW], bf16, tag="wu",
                                          name=f"wub_{j}_{fb}")
                        w1tiles[j][fb] = (wgb, wub)
                        nc.gpsimd.dma_start(wgb[:], wg_d[j, fb])
                        nc.gpsimd.dma_start(wub[:], wu_d[j, fb])
                    wgb, wub = w1tiles[j][fb]
                    for fs in range(FT_PER_FB[fb]):
                        ft = fb * 4 + fs
                        for ci, (c0, cn) in enumerate(zip(jstarts, jchunks)):
                            pg = ppool.tile([P, 512], f32, tag=f"g{ci}")
                            pu = ppool.tile([P, 512], f32, tag=f"u{ci}")
                            for ko in range(KD):
                                nc.tensor.matmul(
                                    pg[:, :cn],
                                    wgb[:, ko, fs * P:(fs + 1) * P],
                                    xt[:, ko, soff + c0:soff + c0 + cn],
                                    start=(ko == 0), stop=(ko == KD - 1))
                            for ko in range(KD):
                                nc.tensor.matmul(
                                    pu[:, :cn],
                                    wub[:, ko, fs * P:(fs + 1) * P],
                                    xt[:, ko, soff + c0:soff + c0 + cn],
                                    start=(ko == 0), stop=(ko == KD - 1))
                            a_sl = at[:, ft, c0:c0 + cn]
                            nc.scalar.activation(
                                a_sl, pg[:, :cn],
                                mybir.ActivationFunctionType.Silu)
                            nc.vector.tensor_tensor(a_sl, a_sl, pu[:, :cn],
                                                    mult)

                # ---- phase 2: dT = Wd.T @ aT (partial over F-quarter),
                #      out = dT * w ----
                for dg in range(NDG):
                    wdb = wd2pool.tile([P, KFQ, P * DG], bf16, tag="wd",
                                       name=f"wdb_{j}_{dg}")
                    nc.gpsimd.dma_start(wdb[:], wd_d[j, dg])
                    pds = [[None] * len(jchunks) for _ in range(DG)]
                    ptags = [["g", "g"], ["u", "u"]]
                    for ds in range(DG):
                        for ci in range(len(jchunks)):
                            pds[ds][ci] = ppool.tile(
                                [P, 512], f32, tag=f"{ptags[ds][ci]}{ci}",
                                name=f"pd_{j}_{dg}_{ds}_{ci}")
                    for fk in range(KFQ):
                        for ds in range(DG):
                            for ci, (c0, cn) in enumerate(
                                    zip(jstarts, jchunks)):
                                nc.tensor.matmul(
                                    pds[ds][ci][:, :cn],
                                    wdb[:, fk, ds * P:(ds + 1) * P],
                                    at[:, fk, c0:c0 + cn],
                                    start=(fk == 0), stop=(fk == KFQ - 1))
                    for ds in range(DG):
                        dt_idx = dg * DG + ds
                        for ci, (c0, cn) in enumerate(zip(jstarts, jchunks)):
                            ot = outpool.tile([P, 512], f32, tag="ot")
                            nc.vector.tensor_tensor(
                                ot[:, :cn], pds[ds][ci][:, :cn],
                                wrep[:, soff + c0:soff + c0 + cn], mult)
                            nc.sync.dma_start(
                                out_d[dt_idx * P:(dt_idx + 1) * P,
                                      soff + c0:soff + c0 + cn],
                                ot[:, :cn])

    nc.compile()
    return nc


def _swizzle_w1(wq):
    """[D, FQ] bf16 -> [NFB, P, KD, FBW] block-major (last block padded)."""
    wp = np.zeros((D, NFB * FBW), dtype=wq.dtype)
    wp[:, :FQ] = wq
    return np.ascontiguousarray(
        wp.reshape(KD, P, NFB, FBW).transpose(2, 1, 0, 3))


def _swizzle_wd(wq):
    """[FQ, D] bf16 -> [NDG, P, KFQ, P*DG] block-major."""
    return np.ascontiguousarray(
        wq.reshape(KFQ, P, NDG, P * DG).transpose(2, 1, 0, 3))


def kernel(x, gate_tensor, Wg, Wu, Wd):
    global LAST_RESULT
    import ml_dtypes
    from concourse.bass_interp import get_hw_module
    from concourse.bass_utils import run_bass_kernel_spmd

    bf = ml_dtypes.bfloat16
    x = np.ascontiguousarray(np.asarray(x, dtype=np.float32))
    gate_tensor = np.asarray(gate_tensor, dtype=np.float32)
    Wg = np.asarray(Wg, dtype=np.float32)
    Wu = np.asarray(Wu, dtype=np.float32)
    Wd = np.asarray(Wd, dtype=np.float32)

    # ---- router (replicated; tiny: T*D*E flops) ----
    logits = x @ gate_tensor                      # [T, E] fp32
    m = logits.max(axis=-1, keepdims=True)
    p = np.exp(logits - m, dtype=np.float32)
    p /= p.sum(axis=-1, keepdims=True)
    topi = np.argsort(-p, axis=-1, kind="stable")[:, :TOPK]      # [T, K]
    topw = np.take_along_axis(p, topi, axis=-1)
    topw = topw / (topw.sum(axis=-1, keepdims=True) + 1e-20)

    idx = []          # tokens routed to each expert
    wts = []          # their combine weights
    for e in range(E):
        sel = (topi == e)                         # [T, K]; <=1 True per row
        idx.append(np.nonzero(sel.any(axis=-1))[0])
        wts.append(topw[sel].astype(np.float32))  # row-major == token order

    # ---- balanced expert grouping: sort by load desc, alternate ranks ----
    order = sorted(range(E), key=lambda e: -len(idx[e]))
    groups = [[order[2 * j + g] for j in range(NQ)] for g in range(2)]
    slot_sizes = tuple(
        (max(len(idx[groups[0][j]]), len(idx[groups[1][j]])) + 1) // 2 * 2
        for j in range(NQ))
    soffs = [sum(slot_sizes[:j]) for j in range(NQ)]
    cap = sum(slot_sizes)

    if slot_sizes not in _COMPILED:
        _COMPILED[slot_sizes] = _build(slot_sizes)
    nc = _COMPILED[slot_sizes]

    # ---- dispatch: per-core inputs (bf16, pre-swizzled) ----
    x_bf = x.astype(bf)
    Wg_bf = Wg.astype(bf)
    Wu_bf = Wu.astype(bf)
    Wd_bf = Wd.astype(bf)

    in_maps = []
    for g in range(2):
        # tokens + combine weights shared by the group's 4 cores
        xt = np.zeros((P, KD, cap), dtype=bf)
        wr = np.zeros((P, cap), dtype=np.float32)
        for j in range(NQ):
            e = groups[g][j]
            n = len(idx[e])
            xg = x_bf[idx[e]]                      # [n, D]
            xt[:, :, soffs[j]:soffs[j] + n] = (
                xg.T.reshape(KD, P, n).transpose(1, 0, 2))
            wr[:, soffs[j]:soffs[j] + n] = wts[e][None, :]
        for q in range(NQ):
            fsl = slice(q * FQ, (q + 1) * FQ)
            wg = np.stack([_swizzle_w1(Wg_bf[groups[g][j]][:, fsl])
                           for j in range(NQ)])
            wu = np.stack([_swizzle_w1(Wu_bf[groups[g][j]][:, fsl])
                           for j in range(NQ)])
            wd = np.stack([_swizzle_wd(Wd_bf[groups[g][j]][fsl, :])
                           for j in range(NQ)])
            in_maps.append({"xt": xt, "wg": wg, "wu": wu, "wd": wd,
                            "wrep": wr})

    trace = bool(int(os.environ.get("KERNEL_TRACE", "0")))
    old_m = nc.m
    nc.m = get_hw_module(nc.m)
    try:
        try:
            res = run_bass_kernel_spmd(nc, in_maps, core_ids=list(range(E)),
                                       trace=trace)
        except (ImportError, ModuleNotFoundError):
            # tracing requested (e.g. BASS_TRACE in the env) but this image
            # lacks the axon NTFF profile hook -- rerun without tracing
            os.environ["BASS_NEVER_TRACE"] = "1"
            res = run_bass_kernel_spmd(nc, in_maps, core_ids=list(range(E)),
                                       trace=False)
    finally:
        nc.m = old_m
    LAST_RESULT = res

    # ---- combine: tp_reduce over the 4 quarters, then scatter-add ----
    out = np.zeros((T, D), dtype=np.float32)
    for g in range(2):
        acc = res.results[g * NQ]["out_t"].astype(np.float64)
        for q in range(1, NQ):
            acc += res.results[g * NQ + q]["out_t"]
        acc = acc.astype(np.float32)
        for j in range(NQ):
            e = groups[g][j]
            n = len(idx[e])
            out[idx[e]] += acc[:, soffs[j]:soffs[j] + n].T
    return out
